# revision 1
# baseline (speedup 1.0000x reference)
"""Int8RouterLinear TRN2 kernel: out[16384, 64] = x[16384, 4096] @ (W_int8 * scale)^T.

v2 strategy (data-parallel over 8 NeuronCores, 2048 tokens each):
  - Host quantizes x per token: h-tiles k>=8 to int8 (u = rint(x/s_t),
    s_t = absmax_t/127), h-tiles k<8 to fp8-e4m3 of x/s_t. 1 byte/elem
    either way -> 8MB of x per core (vs 14.1MB for the fp16/fp8 mix).
    int8's uniform grid is ~3x more accurate than fp8 for Gaussian x.
  - On device, int8 h-tiles are cast to fp16 (exact: |u| <= 127) split
    across DVE (2x mode, ~1.92 elem/ns/partition) and ACT
    ((N+352)/1.2ns); fp8 tiles feed the PE directly (fp16 lhsT x fp8
    rhs mixed matmul, same speed).
  - PE runs col-tiled: the 2048 tokens form 2 super-chunks of 1024; a
    super-chunk's two 512-token halves run CONCURRENTLY in PE column
    groups 0-63 / 64-127 (tile_position via out base partition), so a
    k-step costs ~216ns for 1024 tokens -> ~14us PE total.
  - PSUM: one [128, 512] f32 bank per super-chunk (half-partitions =
    token halves), accumulated over the 32 h-tiles, then one ACT
    scaled-copy (2^-6, fits fp16) -> [128, 512] fp16 out, DMA'd out.
  - Host post-scales: out = psum_fp16 * 2^6 * s_t * scale_e. Weight
    ships as fp16 (int8 values exact).
  - DMA: x + w + out = 8.75MB/core over both HWDGE rings, blocks
    interleaved in program (k) order so completion tracks the
    cast/matmul consumption order.
"""
import numpy as np

import concourse.mybir as mybir
from concourse import bacc
from concourse.tile import TileContext
from concourse.bass_utils import run_bass_kernel_spmd

TOKENS = 16384
HIDDEN = 4096
EXPERTS = 64
NCORES = 8
TSHARD = TOKENS // NCORES          # 2048 tokens per core
HT = HIDDEN // 128                 # 32 h-tiles of 128
HT8 = 8                            # leading h-tiles in fp8 (no cast)
HTI = HT - HT8                     # trailing h-tiles shipped as int8
NS = 2                             # super-chunks of 1024 tokens
SU = 1024                          # tokens per super-chunk
CH = 512                           # tokens per col-group chunk

F32 = mybir.dt.float32
F16 = mybir.dt.float16
F8 = mybir.dt.float8e4
I8 = mybir.dt.int8

# DMA blocks in program order: (name, ring, kind, s, k0, nk)
#   kind: 'w' weight, 'f8' fp8 x slab, 'i8' int8 x slab
BLOCKS = [
    ("w",    1, "w",  0, 0, 0),
    ("f8a",  0, "f8", 0, 0, 8),
    ("i8a",  1, "i8", 0, 8, 8),
    ("i8b",  0, "i8", 0, 16, 8),
    ("i8c",  1, "i8", 0, 24, 8),
    ("f8b",  0, "f8", 1, 0, 8),
    ("i8d",  1, "i8", 1, 8, 8),
    ("i8e",  0, "i8", 1, 16, 8),
    ("i8f",  1, "i8", 1, 24, 6),
    ("i8g",  0, "i8", 1, 30, 2),
]
# int8 cast ops: (engine, s, k0, nk) in consumption order. DVE ~33 units,
# ACT ~15 units balances DVE 2x vs ACT 1x rates (ACT also does the two
# tail copies and pays the one-time table load).
CASTS = [
    ("v", 0, 8, 5), ("a", 0, 13, 3),
    ("v", 0, 16, 5), ("a", 0, 21, 3),
    ("v", 0, 24, 5), ("a", 0, 29, 3),
    ("v", 1, 8, 6), ("a", 1, 14, 2),
    ("v", 1, 16, 6), ("a", 1, 22, 2),
    ("v", 1, 24, 5), ("a", 1, 29, 1),
    ("a", 1, 30, 1), ("v", 1, 31, 1),
]

_cache = {}


def _build():
    if "nc" in _cache:
        return _cache["nc"]

    nc = bacc.Bacc("TRN2", target_bir_lowering=False, debug=False,
                   num_devices=NCORES)
    x8_d = nc.dram_tensor("x8", [128, NS * HT8, SU], F8, kind="ExternalInput")
    xi_d = nc.dram_tensor("xi", [128, NS * HTI, SU], I8, kind="ExternalInput")
    w_d = nc.dram_tensor("w", [128, HT * EXPERTS], F16, kind="ExternalInput")
    o_d = nc.dram_tensor("out", [128, NS * CH], F16, kind="ExternalOutput")

    with TileContext(nc) as tc:
        with tc.tile_pool(name="consts", bufs=1) as cpool, \
             tc.tile_pool(name="xp", bufs=1) as xpool, \
             tc.tile_pool(name="xf", bufs=1) as fpool, \
             tc.tile_pool(name="pso", bufs=1, space="PSUM") as ppool, \
             tc.tile_pool(name="ost", bufs=1) as opool:
            rings = [nc.sync, nc.scalar]

            src_tiles = {}
            w_sb = cpool.tile([128, HT * EXPERTS], F16)

            for name, ring, kind, s, k0, nk in BLOCKS:
                if kind == "w":
                    rings[ring].dma_start(out=w_sb, in_=w_d.ap())
                    continue
                if kind == "f8":
                    src = x8_d.ap()[:, s * HT8 + k0:s * HT8 + k0 + nk, :]
                    t = xpool.tile([128, nk * SU], F8, name=name, tag=name)
                else:
                    src = xi_d.ap()[:, s * HTI + (k0 - HT8):s * HTI + (k0 - HT8) + nk, :]
                    t = xpool.tile([128, nk * SU], I8, name=name, tag=name)
                rings[ring].dma_start(out=t, in_=src)
                for j in range(nk):
                    src_tiles[(s, k0 + j)] = (t, j)

            w_v = w_sb.rearrange("p (k e) -> p k e", e=EXPERTS)

            # cast int8 units -> fp16 tiles
            f16_units = {}
            for eng, s, k0, nk in CASTS:
                it, j0 = src_tiles[(s, k0)]
                iv = it.rearrange("p (u t) -> p u t", t=SU)
                ft = fpool.tile([128, nk * SU], F16, name=f"c{eng}{s}k{k0}",
                                tag=f"c{eng}{s}k{k0}")
                if eng == "v":
                    nc.vector.tensor_copy(ft, iv[:, j0:j0 + nk, :])
                else:
                    nc.scalar.copy(ft, iv[:, j0:j0 + nk, :])
                for j in range(nk):
                    f16_units[(s, k0 + j)] = (ft, j)

            # PE: col-tiled accumulation, one PSUM bank per super-chunk
            pps = [ppool.tile([128, CH], F32, name=f"pp{s}", tag=f"pp{s}")
                   for s in range(NS)]
            ots = [opool.tile([128, CH], F16, name=f"ot{s}", tag=f"ot{s}")
                   for s in range(NS)]
            for s in range(NS):
                pp = pps[s]
                for k in range(HT):
                    if k < HT8:
                        t, j = src_tiles[(s, k)]
                    else:
                        t, j = f16_units[(s, k)]
                    tv = t.rearrange("p (u t) -> p u t", t=SU)
                    wt = w_v[:, k, :]
                    nc.tensor.matmul(pp[0:64, :], wt, tv[:, j, 0:CH],
                                     start=(k == 0), stop=(k == HT - 1))
                    nc.tensor.matmul(pp[64:128, :], wt, tv[:, j, CH:SU],
                                     start=(k == 0), stop=(k == HT - 1))
                # tail: one scaled fp32->fp16 copy (2^-6 keeps |v| < 2^16/6.4)
                nc.scalar.mul(ots[s], pp, 0.015625)
                rings[s % 2].dma_start(out=o_d.ap()[:, s * CH:(s + 1) * CH],
                                       in_=ots[s])

    nc.compile()
    _cache["nc"] = nc
    return nc


def _prep_w(weights_int8, scales):
    """[64, 4096] int8-valued weights -> [128, HT*EXPERTS] fp16 with
    w_arr[p, k*64 + e] = W[e, 128k + p] (int values, exact in fp16)."""
    wt = weights_int8.astype(np.float32).T.astype(np.float16)  # [H, E]
    arr = wt.reshape(HT, 128, EXPERTS).transpose(1, 0, 2)
    return np.ascontiguousarray(arr).reshape(128, HT * EXPERTS)


def _prep_x(x):
    """Quantize + transpose x into per-core (x8, xi) plus token scales:
    x8[p, s*8+k, t]   = fp8((x[T0 + s*1024 + t, 128k + p]) / s_tok)   k<8
    xi[p, s*24+k', t] = rint(x[T0 + s*1024 + t, 128(k'+8) + p] / s_tok)
    """
    f8np = mybir.dt.np(F8)
    s_tok = np.abs(x).max(axis=1) / 127.0            # [TOKENS]
    s_tok = np.maximum(s_tok, 1e-12).astype(np.float32)
    xs = x / s_tok[:, None]                          # |xs| <= 127
    H8 = HT8 * 128
    x8 = xs[:, :H8].astype(f8np)
    xi = np.clip(np.rint(xs[:, H8:]), -127, 127).astype(np.int8)
    xt8 = np.empty((H8, TOKENS), dtype=f8np)
    xti = np.empty((HIDDEN - H8, TOKENS), dtype=np.int8)
    blk = 512
    for i in range(0, TOKENS, blk):
        xt8[:, i:i + blk] = x8[i:i + blk].T
        xti[:, i:i + blk] = xi[i:i + blk].T
    shards = []
    for c in range(NCORES):
        sl = slice(c * TSHARD, (c + 1) * TSHARD)
        # [H=k*128, 2048=NS*SU] -> [128, NS, k, SU]
        a8 = xt8[:, sl].reshape(HT8, 128, NS, SU).transpose(1, 2, 0, 3)
        ai = xti[:, sl].reshape(HTI, 128, NS, SU).transpose(1, 2, 0, 3)
        shards.append((
            np.ascontiguousarray(a8).reshape(128, NS * HT8, SU),
            np.ascontiguousarray(ai).reshape(128, NS * HTI, SU),
        ))
    return shards, s_tok


def kernel(x, weights_int8, scales):
    nc = _build()
    x = np.ascontiguousarray(np.asarray(x), dtype=np.float32)
    warr = _prep_w(np.asarray(weights_int8), np.asarray(scales))
    shards, s_tok = _prep_x(x)
    in_maps = [{"x8": shards[c][0], "xi": shards[c][1], "w": warr}
               for c in range(NCORES)]
    res = run_bass_kernel_spmd(nc, in_maps, core_ids=list(range(NCORES)))
    scales_f = np.asarray(scales, dtype=np.float64)
    out = np.empty((TOKENS, EXPERTS), dtype=np.float64)
    for c in range(NCORES):
        o = res.results[c]["out"].astype(np.float64)      # [128, NS*CH]
        o = o.reshape(2, 64, NS, CH)                      # [chunk, e, s, t]
        for s in range(NS):
            for ch in range(2):
                t0 = c * TSHARD + s * SU + ch * CH
                out[t0:t0 + CH] = o[ch, :, s, :].T
    out *= 64.0 * s_tok[:, None].astype(np.float64)
    out *= scales_f[None, :]
    return np.ascontiguousarray(out, dtype=np.float32)
